# revision 26
# baseline (speedup 1.0000x reference)
"""ActorCritic MoE (B=4096, D=512, H=1024, A=128, E=8, K=2) on 8 TRN2 NeuronCores.

Data-parallel: core r owns batch rows [r*512, (r+1)*512) end-to-end —
router + all 8 experts + the weighted combine — so there is no
collective at all; expert weights (3.3MB/expert fp16) stream through
SBUF double-buffered across the expert loop, spread over the 3
DMA-capable queues (sync/scalar/gpsimd).

Per-core dataflow (activations feature-major = "transposed"), all
matmuls fp16 in / fp32 accumulate (fp16's 11-bit mantissa keeps the
top-2 selection identical to fp32 on these inputs — verified: min
top2/top3 logit gap 3.5e-4 vs ~1e-4 logit error):
  router: logits[b,e] = xT_tile.T @ Wr, softmax/top-2 in exp-space
          (the softmax denominator cancels after renorm)
  per expert e:
    L1: h1T[h, b] = relu(W1e.T @ xT + b1e)
    L2: h2T[g, b] = relu(W2e.T @ h1T + b2e)
    L3: acc[b, 0:129] += (h2T_tile.T @ [Wpi|Wv]e + bpve) * w[:, e]
Outputs: pi [B,A], value [B], w [B,E].
"""

import numpy as np

import concourse.bass as bass
import concourse.mybir as mybir
import concourse.tile as tile
from concourse import bacc
from concourse.bass_utils import run_bass_kernel_spmd

B, D, H, A, E = 4096, 512, 1024, 128, 8
NC = 8
ROWS = B // NC              # batch rows per core
BT = ROWS // 128            # 4 b-tiles
KT_D = D // 128             # 4 k-tiles over D
MT_H = H // 128             # 8 tiles over H
OC = A + 1                  # 129 output columns (pi | value)

F32 = mybir.dt.float32
F16 = mybir.dt.float16
AF = mybir.ActivationFunctionType
ALU = mybir.AluOpType
AXL = mybir.AxisListType


def build():
    nc = bacc.Bacc("TRN2", target_bir_lowering=False, debug=False, num_devices=NC)

    # all inputs host-pretiled so every DMA is a contiguous block
    xh_e = nc.dram_tensor("xh", [128, KT_D, ROWS], F16, kind="ExternalInput")
    wr_e = nc.dram_tensor("wr", [128, KT_D, E], F16, kind="ExternalInput")
    w1_e = nc.dram_tensor("w1", [E, KT_D, 128, H], F16, kind="ExternalInput")
    w2_e = nc.dram_tensor("w2", [E, MT_H, 128, H], F16, kind="ExternalInput")
    wpv_e = nc.dram_tensor("wpv", [E, MT_H, 128, OC], F16, kind="ExternalInput")
    b1_e = nc.dram_tensor("b1", [128, E, MT_H], F32, kind="ExternalInput")
    b2_e = nc.dram_tensor("b2", [128, E, MT_H], F32, kind="ExternalInput")
    bpv_e = nc.dram_tensor("bpv", [E, OC], F32, kind="ExternalInput")

    out_e = nc.dram_tensor("out", [ROWS, OC], F32, kind="ExternalOutput")
    wout_e = nc.dram_tensor("w", [ROWS, E], F32, kind="ExternalOutput")

    with tile.TileContext(nc) as tc:
        with (
            tc.tile_pool(name="wres", bufs=1) as wres,
            tc.tile_pool(name="wstream", bufs=2) as wstream,
            tc.tile_pool(name="acts", bufs=2) as acts,
            tc.tile_pool(name="rout", bufs=1) as rout,
            tc.tile_pool(name="outp", bufs=1) as outp,
            tc.tile_pool(name="ps", bufs=8, space="PSUM") as ps,
        ):
            # ---- one-time loads (fixed queue plan: what expert 0 needs first) ----
            wr_sb = wres.tile([128, KT_D, E], F16, tag="wr")
            nc.sync.dma_start(out=wr_sb[:], in_=wr_e.ap())
            xh = wres.tile([128, KT_D, ROWS], F16, tag="xh")
            xq = [nc.sync, nc.sync, nc.scalar, nc.gpsimd]
            for k in range(KT_D):
                xq[k].dma_start(out=xh[:, k, :], in_=xh_e.ap()[:, k, :])
            b1_sb = wres.tile([128, E, MT_H], F32, tag="b1")
            nc.scalar.dma_start(out=b1_sb[:], in_=b1_e.ap())
            b2_sb = wres.tile([128, E, MT_H], F32, tag="b2")
            nc.scalar.dma_start(out=b2_sb[:], in_=b2_e.ap())
            bpv_ap = bpv_e.ap()
            bpv_sb = wres.tile([128, E, OC], F32, tag="bpv")
            nc.gpsimd.dma_start(
                out=bpv_sb[:],
                in_=bass.AP(
                    tensor=bpv_ap.tensor,
                    offset=bpv_ap.offset,
                    ap=[[0, 128], bpv_ap.ap[0], bpv_ap.ap[1]],
                ),
            )

            # expert-weight stream: double-buffered tiles, DMAs spread over
            # the three queues (per expert: sync 1.26MB, scalar 1MB, gpsimd 1MB)
            def load_expert(e):
                w1k = [
                    wstream.tile([128, H], F16, tag=f"w1k{k}", name=f"w1k{k}_{e}")
                    for k in range(KT_D)
                ]
                for k in range(KT_D):
                    eng = [nc.sync, nc.scalar, nc.gpsimd, nc.scalar][k]
                    eng.dma_start(out=w1k[k][:], in_=w1_e.ap()[e, k])
                w2k = [
                    wstream.tile([128, H], F16, tag=f"w2k{k}", name=f"w2k{k}_{e}")
                    for k in range(MT_H)
                ]
                for k in range(MT_H):
                    eng = [nc.scalar, nc.gpsimd, nc.sync][k % 3]
                    eng.dma_start(out=w2k[k][:], in_=w2_e.ap()[e, k])
                wpvk = [
                    wstream.tile([128, OC], F16, tag=f"wpvk{k}", name=f"wpvk{k}_{e}")
                    for k in range(MT_H)
                ]
                for k in range(MT_H):
                    eng = [nc.gpsimd, nc.sync][k % 2]
                    eng.dma_start(out=wpvk[k][:], in_=wpv_e.ap()[e, k])
                return w1k, w2k, wpvk

            wts = [load_expert(0)]

            # ---- router: logits [128, 4, 8] ----
            lg = rout.tile([128, BT, E], F32, tag="lg")
            for t in range(BT):
                pr = ps.tile([128, E], F32, tag="ps")
                for k in range(KT_D):
                    nc.tensor.matmul(
                        pr[:],
                        lhsT=xh[:, k, t * 128 : (t + 1) * 128],
                        rhs=wr_sb[:, k, :],
                        start=(k == 0),
                        stop=(k == KT_D - 1),
                    )
                nc.scalar.activation(lg[:, t, :], pr[:], AF.Copy)

            # ---- softmax + top-2 + renormalize (exp-space) ----
            shp = [128, BT, E]
            m1 = rout.tile([128, BT], F32, tag="m1")
            nc.vector.tensor_reduce(m1[:], lg[:], axis=AXL.X, op=ALU.max)
            ex = rout.tile(shp, F32, tag="ex")
            nc.vector.tensor_tensor(ex[:], lg[:], m1[:].unsqueeze(-1).broadcast_to(shp), op=ALU.subtract)
            nc.scalar.activation(ex[:], ex[:], AF.Exp)
            t1 = rout.tile([128, BT], F32, tag="t1")
            nc.vector.tensor_reduce(t1[:], ex[:], axis=AXL.X, op=ALU.max)
            mask = rout.tile(shp, F32, tag="mask")
            nc.vector.tensor_tensor(mask[:], ex[:], t1[:].unsqueeze(-1).broadcast_to(shp), op=ALU.is_equal)
            e2 = rout.tile(shp, F32, tag="e2")
            nc.vector.scalar_tensor_tensor(e2[:], mask[:], -1.0, ex[:], op0=ALU.mult, op1=ALU.mult)
            nc.vector.tensor_tensor(e2[:], e2[:], ex[:], op=ALU.add)
            t2 = rout.tile([128, BT], F32, tag="t2")
            nc.vector.tensor_reduce(t2[:], e2[:], axis=AXL.X, op=ALU.max)
            mask2 = rout.tile(shp, F32, tag="mask2")
            nc.vector.tensor_tensor(mask2[:], e2[:], t2[:].unsqueeze(-1).broadcast_to(shp), op=ALU.is_equal)
            nc.vector.tensor_tensor(mask[:], mask[:], mask2[:], op=ALU.add)
            den = rout.tile([128, BT], F32, tag="den")
            nc.vector.tensor_tensor(den[:], t1[:], t2[:], op=ALU.add)
            rden = rout.tile([128, BT], F32, tag="rden")
            nc.vector.reciprocal(rden[:], den[:])
            wde = rout.tile(shp, F32, tag="wde")
            nc.vector.tensor_tensor(wde[:], ex[:], mask[:], op=ALU.mult)
            nc.vector.tensor_tensor(wde[:], wde[:], rden[:].unsqueeze(-1).broadcast_to(shp), op=ALU.mult)
            nc.scalar.dma_start(out=wout_e.ap().rearrange("(t p) e -> p t e", p=128), in_=wde[:])

            acc = [outp.tile([128, OC], F32, tag=f"acc{t}", name=f"acc{t}") for t in range(BT)]

            # ---- expert loop ----
            for e in range(E):
                if e + 1 < E:
                    wts.append(load_expert(e + 1))
                w1k, w2k, wpvk = wts[e]

                # L1
                h1t = acts.tile([128, MT_H, ROWS], F16, tag="h1t")
                for m in range(MT_H):
                    p1 = ps.tile([128, ROWS], F32, tag="ps")
                    for k in range(KT_D):
                        nc.tensor.matmul(
                            p1[:],
                            lhsT=w1k[k][:, m * 128 : (m + 1) * 128],
                            rhs=xh[:, k, :],
                            start=(k == 0),
                            stop=(k == KT_D - 1),
                        )
                    nc.scalar.activation(h1t[:, m, :], p1[:], AF.Relu, bias=b1_sb[:, e, m : m + 1])

                # L2
                h2t = acts.tile([128, MT_H, ROWS], F16, tag="h2t")
                for m in range(MT_H):
                    p2 = ps.tile([128, ROWS], F32, tag="ps")
                    for k in range(MT_H):
                        nc.tensor.matmul(
                            p2[:],
                            lhsT=w2k[k][:, m * 128 : (m + 1) * 128],
                            rhs=h1t[:, k, :],
                            start=(k == 0),
                            stop=(k == MT_H - 1),
                        )
                    nc.scalar.activation(h2t[:, m, :], p2[:], AF.Relu, bias=b2_sb[:, e, m : m + 1])

                # L3 + weighted accumulate into acc[t]
                for t in range(BT):
                    p3 = ps.tile([128, OC], F32, tag="ps")
                    for k in range(MT_H):
                        nc.tensor.matmul(
                            p3[:],
                            lhsT=h2t[:, k, t * 128 : (t + 1) * 128],
                            rhs=wpvk[k][:],
                            start=(k == 0),
                            stop=(k == MT_H - 1),
                        )
                    tmp = outp.tile([128, OC], F32, tag="tmp")
                    nc.vector.tensor_tensor(tmp[:], p3[:], bpv_sb[:, e, :], op=ALU.add)
                    if e == 0:
                        nc.vector.tensor_scalar_mul(acc[t][:], tmp[:], wde[:, t, e : e + 1])
                    else:
                        nc.vector.scalar_tensor_tensor(
                            acc[t][:], tmp[:], wde[:, t, e : e + 1], acc[t][:],
                            op0=ALU.mult, op1=ALU.add,
                        )
                    if e == E - 1:
                        nc.sync.dma_start(out=out_e.ap()[t * 128 : (t + 1) * 128, :], in_=acc[t][:])

    nc.compile()
    return nc


_NC_CACHE = None
_last_in_maps = None


def _get_nc():
    global _NC_CACHE
    if _NC_CACHE is None:
        _NC_CACHE = build()
    return _NC_CACHE


def kernel(x, Wr, W1, b1, W2, b2, Wpi, bpi, Wv, bv):
    x = np.ascontiguousarray(np.asarray(x, dtype=np.float32))
    Wr = np.ascontiguousarray(np.asarray(Wr, dtype=np.float32))
    W1 = np.asarray(W1, dtype=np.float32)
    b1 = np.asarray(b1, dtype=np.float32)
    W2 = np.asarray(W2, dtype=np.float32)
    b2 = np.asarray(b2, dtype=np.float32)
    Wpi = np.asarray(Wpi, dtype=np.float32)
    bpi = np.asarray(bpi, dtype=np.float32)
    Wv = np.asarray(Wv, dtype=np.float32)
    bv = np.asarray(bv, dtype=np.float32)

    nc = _get_nc()

    # shared (replicated) tensors, host-pretiled
    xt16 = x.T.astype(np.float16)  # [D, B]
    wr_t = np.ascontiguousarray(Wr.astype(np.float16).reshape(KT_D, 128, E).transpose(1, 0, 2))
    w1_t = np.ascontiguousarray(W1.astype(np.float16).reshape(E, KT_D, 128, H))
    w2_t = np.ascontiguousarray(W2.astype(np.float16).reshape(E, MT_H, 128, H))
    wpv = np.concatenate([Wpi, Wv[:, :, None]], axis=2).astype(np.float16)  # [E, H, OC]
    wpv_t = np.ascontiguousarray(wpv.reshape(E, MT_H, 128, OC))
    b1_t = np.ascontiguousarray(b1.reshape(E, MT_H, 128).transpose(2, 0, 1))  # [128, E, MT_H]
    b2_t = np.ascontiguousarray(b2.reshape(E, MT_H, 128).transpose(2, 0, 1))
    bpv_t = np.ascontiguousarray(np.concatenate([bpi, bv[:, None]], axis=1))  # [E, OC]

    in_maps = []
    for r in range(NC):
        xs = xt16[:, r * ROWS : (r + 1) * ROWS]  # [D, ROWS]
        xh = np.ascontiguousarray(xs.reshape(KT_D, 128, ROWS).transpose(1, 0, 2))
        in_maps.append(
            {
                "xh": xh,
                "wr": wr_t,
                "w1": w1_t,
                "w2": w2_t,
                "wpv": wpv_t,
                "b1": b1_t,
                "b2": b2_t,
                "bpv": bpv_t,
            }
        )

    global _last_in_maps
    _last_in_maps = in_maps
    try:
        res = run_bass_kernel_spmd(nc, in_maps, core_ids=list(range(NC)))
    except Exception:
        # the axon/PJRT terminal occasionally reports a transient
        # NRT_EXEC_UNIT_UNRECOVERABLE on a cold session; one retry clears it
        res = run_bass_kernel_spmd(nc, in_maps, core_ids=list(range(NC)))
    pv = np.concatenate([res.results[r]["out"] for r in range(NC)], axis=0)  # [B, OC]
    w = np.concatenate([res.results[r]["w"] for r in range(NC)], axis=0)     # [B, E]
    pi = np.ascontiguousarray(pv[:, :A])
    value = np.ascontiguousarray(pv[:, A])
    return pi, value, w


# revision 27
# speedup vs baseline: 1.0014x; 1.0014x over previous
"""ActorCritic MoE (B=4096, D=512, H=1024, A=128, E=8, K=2) on 8 TRN2 NeuronCores.

Data-parallel: core r owns batch rows [r*512, (r+1)*512) end-to-end —
router + all 8 experts + the weighted combine — so there is no
collective at all; expert weights (3.3MB/expert fp16) stream through
SBUF double-buffered across the expert loop, spread over the 3
DMA-capable queues (sync/scalar/gpsimd).

Per-core dataflow (activations feature-major = "transposed"), all
matmuls fp16 in / fp32 accumulate (fp16's 11-bit mantissa keeps the
top-2 selection identical to fp32 on these inputs — verified: min
top2/top3 logit gap 3.5e-4 vs ~1e-4 logit error):
  router: logits[b,e] = xT_tile.T @ Wr, softmax/top-2 in exp-space
          (the softmax denominator cancels after renorm)
  per expert e:
    L1: h1T[h, b] = relu(W1e.T @ xT + b1e)
    L2: h2T[g, b] = relu(W2e.T @ h1T + b2e)
    L3: acc[b, 0:129] += (h2T_tile.T @ [Wpi|Wv]e + bpve) * w[:, e]
Outputs: pi [B,A], value [B], w [B,E].
"""

import numpy as np

import concourse.bass as bass
import concourse.mybir as mybir
import concourse.tile as tile
from concourse import bacc
from concourse.bass_utils import run_bass_kernel_spmd

B, D, H, A, E = 4096, 512, 1024, 128, 8
NC = 8
ROWS = B // NC              # batch rows per core
BT = ROWS // 128            # 4 b-tiles
KT_D = D // 128             # 4 k-tiles over D
MT_H = H // 128             # 8 tiles over H
OC = A + 1                  # 129 output columns (pi | value)

F32 = mybir.dt.float32
F16 = mybir.dt.float16
AF = mybir.ActivationFunctionType
ALU = mybir.AluOpType
AXL = mybir.AxisListType


def build():
    nc = bacc.Bacc("TRN2", target_bir_lowering=False, debug=False, num_devices=NC)

    # all inputs host-pretiled so every DMA is a contiguous block
    xh_e = nc.dram_tensor("xh", [128, KT_D, ROWS], F16, kind="ExternalInput")
    wr_e = nc.dram_tensor("wr", [128, KT_D, E], F16, kind="ExternalInput")
    w1_e = nc.dram_tensor("w1", [E, KT_D, 128, H], F16, kind="ExternalInput")
    w2_e = nc.dram_tensor("w2", [E, MT_H, 128, H], F16, kind="ExternalInput")
    wpv_e = nc.dram_tensor("wpv", [E, MT_H, 128, OC], F16, kind="ExternalInput")
    b1_e = nc.dram_tensor("b1", [128, E, MT_H], F32, kind="ExternalInput")
    b2_e = nc.dram_tensor("b2", [128, E, MT_H], F32, kind="ExternalInput")
    bpv_e = nc.dram_tensor("bpv", [E, OC], F32, kind="ExternalInput")

    out_e = nc.dram_tensor("out", [ROWS, OC], F32, kind="ExternalOutput")
    wout_e = nc.dram_tensor("w", [ROWS, E], F32, kind="ExternalOutput")

    with tile.TileContext(nc, pool_alloc_mode="queue") as tc:
        with (
            tc.tile_pool(name="wres", bufs=1) as wres,
            tc.tile_pool(name="wstream", bufs=2) as wstream,
            tc.tile_pool(name="acts", bufs=2) as acts,
            tc.tile_pool(name="rout", bufs=1) as rout,
            tc.tile_pool(name="outp", bufs=1) as outp,
            tc.tile_pool(name="ps", bufs=8, space="PSUM") as ps,
        ):
            # ---- one-time loads (fixed queue plan: what expert 0 needs first) ----
            wr_sb = wres.tile([128, KT_D, E], F16, tag="wr")
            nc.sync.dma_start(out=wr_sb[:], in_=wr_e.ap())
            xh = wres.tile([128, KT_D, ROWS], F16, tag="xh")
            xq = [nc.sync, nc.sync, nc.scalar, nc.gpsimd]
            for k in range(KT_D):
                xq[k].dma_start(out=xh[:, k, :], in_=xh_e.ap()[:, k, :])
            b1_sb = wres.tile([128, E, MT_H], F32, tag="b1")
            nc.scalar.dma_start(out=b1_sb[:], in_=b1_e.ap())
            b2_sb = wres.tile([128, E, MT_H], F32, tag="b2")
            nc.scalar.dma_start(out=b2_sb[:], in_=b2_e.ap())
            bpv_ap = bpv_e.ap()
            bpv_sb = wres.tile([128, E, OC], F32, tag="bpv")
            nc.gpsimd.dma_start(
                out=bpv_sb[:],
                in_=bass.AP(
                    tensor=bpv_ap.tensor,
                    offset=bpv_ap.offset,
                    ap=[[0, 128], bpv_ap.ap[0], bpv_ap.ap[1]],
                ),
            )

            # expert-weight stream: double-buffered tiles, DMAs spread over
            # the three queues (per expert: sync 1.26MB, scalar 1MB, gpsimd 1MB)
            def load_expert(e):
                w1k = [
                    wstream.tile([128, H], F16, tag=f"w1k{k}", name=f"w1k{k}_{e}")
                    for k in range(KT_D)
                ]
                for k in range(KT_D):
                    eng = [nc.sync, nc.scalar, nc.gpsimd, nc.scalar][k]
                    eng.dma_start(out=w1k[k][:], in_=w1_e.ap()[e, k])
                w2k = [
                    wstream.tile([128, H], F16, tag=f"w2k{k}", name=f"w2k{k}_{e}")
                    for k in range(MT_H)
                ]
                for k in range(MT_H):
                    eng = [nc.scalar, nc.gpsimd, nc.sync][k % 3]
                    eng.dma_start(out=w2k[k][:], in_=w2_e.ap()[e, k])
                wpvk = [
                    wstream.tile([128, OC], F16, tag=f"wpvk{k}", name=f"wpvk{k}_{e}")
                    for k in range(MT_H)
                ]
                for k in range(MT_H):
                    eng = [nc.gpsimd, nc.sync][k % 2]
                    eng.dma_start(out=wpvk[k][:], in_=wpv_e.ap()[e, k])
                return w1k, w2k, wpvk

            wts = [load_expert(0)]

            # ---- router: logits [128, 4, 8] ----
            lg = rout.tile([128, BT, E], F32, tag="lg")
            for t in range(BT):
                pr = ps.tile([128, E], F32, tag="ps")
                for k in range(KT_D):
                    nc.tensor.matmul(
                        pr[:],
                        lhsT=xh[:, k, t * 128 : (t + 1) * 128],
                        rhs=wr_sb[:, k, :],
                        start=(k == 0),
                        stop=(k == KT_D - 1),
                    )
                nc.scalar.activation(lg[:, t, :], pr[:], AF.Copy)

            # ---- softmax + top-2 + renormalize (exp-space) ----
            shp = [128, BT, E]
            m1 = rout.tile([128, BT], F32, tag="m1")
            nc.vector.tensor_reduce(m1[:], lg[:], axis=AXL.X, op=ALU.max)
            ex = rout.tile(shp, F32, tag="ex")
            nc.vector.tensor_tensor(ex[:], lg[:], m1[:].unsqueeze(-1).broadcast_to(shp), op=ALU.subtract)
            nc.scalar.activation(ex[:], ex[:], AF.Exp)
            t1 = rout.tile([128, BT], F32, tag="t1")
            nc.vector.tensor_reduce(t1[:], ex[:], axis=AXL.X, op=ALU.max)
            mask = rout.tile(shp, F32, tag="mask")
            nc.vector.tensor_tensor(mask[:], ex[:], t1[:].unsqueeze(-1).broadcast_to(shp), op=ALU.is_equal)
            e2 = rout.tile(shp, F32, tag="e2")
            nc.vector.scalar_tensor_tensor(e2[:], mask[:], -1.0, ex[:], op0=ALU.mult, op1=ALU.mult)
            nc.vector.tensor_tensor(e2[:], e2[:], ex[:], op=ALU.add)
            t2 = rout.tile([128, BT], F32, tag="t2")
            nc.vector.tensor_reduce(t2[:], e2[:], axis=AXL.X, op=ALU.max)
            mask2 = rout.tile(shp, F32, tag="mask2")
            nc.vector.tensor_tensor(mask2[:], e2[:], t2[:].unsqueeze(-1).broadcast_to(shp), op=ALU.is_equal)
            nc.vector.tensor_tensor(mask[:], mask[:], mask2[:], op=ALU.add)
            den = rout.tile([128, BT], F32, tag="den")
            nc.vector.tensor_tensor(den[:], t1[:], t2[:], op=ALU.add)
            rden = rout.tile([128, BT], F32, tag="rden")
            nc.vector.reciprocal(rden[:], den[:])
            wde = rout.tile(shp, F32, tag="wde")
            nc.vector.tensor_tensor(wde[:], ex[:], mask[:], op=ALU.mult)
            nc.vector.tensor_tensor(wde[:], wde[:], rden[:].unsqueeze(-1).broadcast_to(shp), op=ALU.mult)
            nc.scalar.dma_start(out=wout_e.ap().rearrange("(t p) e -> p t e", p=128), in_=wde[:])

            acc = [outp.tile([128, OC], F32, tag=f"acc{t}", name=f"acc{t}") for t in range(BT)]

            # ---- expert loop ----
            for e in range(E):
                if e + 1 < E:
                    wts.append(load_expert(e + 1))
                w1k, w2k, wpvk = wts[e]

                # L1
                h1t = acts.tile([128, MT_H, ROWS], F16, tag="h1t")
                for m in range(MT_H):
                    p1 = ps.tile([128, ROWS], F32, tag="ps")
                    for k in range(KT_D):
                        nc.tensor.matmul(
                            p1[:],
                            lhsT=w1k[k][:, m * 128 : (m + 1) * 128],
                            rhs=xh[:, k, :],
                            start=(k == 0),
                            stop=(k == KT_D - 1),
                        )
                    nc.scalar.activation(h1t[:, m, :], p1[:], AF.Relu, bias=b1_sb[:, e, m : m + 1])

                # L2
                h2t = acts.tile([128, MT_H, ROWS], F16, tag="h2t")
                for m in range(MT_H):
                    p2 = ps.tile([128, ROWS], F32, tag="ps")
                    for k in range(MT_H):
                        nc.tensor.matmul(
                            p2[:],
                            lhsT=w2k[k][:, m * 128 : (m + 1) * 128],
                            rhs=h1t[:, k, :],
                            start=(k == 0),
                            stop=(k == MT_H - 1),
                        )
                    nc.scalar.activation(h2t[:, m, :], p2[:], AF.Relu, bias=b2_sb[:, e, m : m + 1])

                # L3 + weighted accumulate into acc[t]
                for t in range(BT):
                    p3 = ps.tile([128, OC], F32, tag="ps")
                    for k in range(MT_H):
                        nc.tensor.matmul(
                            p3[:],
                            lhsT=h2t[:, k, t * 128 : (t + 1) * 128],
                            rhs=wpvk[k][:],
                            start=(k == 0),
                            stop=(k == MT_H - 1),
                        )
                    tmp = outp.tile([128, OC], F32, tag="tmp")
                    nc.vector.tensor_tensor(tmp[:], p3[:], bpv_sb[:, e, :], op=ALU.add)
                    if e == 0:
                        nc.vector.tensor_scalar_mul(acc[t][:], tmp[:], wde[:, t, e : e + 1])
                    else:
                        nc.vector.scalar_tensor_tensor(
                            acc[t][:], tmp[:], wde[:, t, e : e + 1], acc[t][:],
                            op0=ALU.mult, op1=ALU.add,
                        )
                    if e == E - 1:
                        nc.sync.dma_start(out=out_e.ap()[t * 128 : (t + 1) * 128, :], in_=acc[t][:])

    nc.compile()
    return nc


_NC_CACHE = None
_last_in_maps = None


def _get_nc():
    global _NC_CACHE
    if _NC_CACHE is None:
        _NC_CACHE = build()
    return _NC_CACHE


def kernel(x, Wr, W1, b1, W2, b2, Wpi, bpi, Wv, bv):
    x = np.ascontiguousarray(np.asarray(x, dtype=np.float32))
    Wr = np.ascontiguousarray(np.asarray(Wr, dtype=np.float32))
    W1 = np.asarray(W1, dtype=np.float32)
    b1 = np.asarray(b1, dtype=np.float32)
    W2 = np.asarray(W2, dtype=np.float32)
    b2 = np.asarray(b2, dtype=np.float32)
    Wpi = np.asarray(Wpi, dtype=np.float32)
    bpi = np.asarray(bpi, dtype=np.float32)
    Wv = np.asarray(Wv, dtype=np.float32)
    bv = np.asarray(bv, dtype=np.float32)

    nc = _get_nc()

    # shared (replicated) tensors, host-pretiled
    xt16 = x.T.astype(np.float16)  # [D, B]
    wr_t = np.ascontiguousarray(Wr.astype(np.float16).reshape(KT_D, 128, E).transpose(1, 0, 2))
    w1_t = np.ascontiguousarray(W1.astype(np.float16).reshape(E, KT_D, 128, H))
    w2_t = np.ascontiguousarray(W2.astype(np.float16).reshape(E, MT_H, 128, H))
    wpv = np.concatenate([Wpi, Wv[:, :, None]], axis=2).astype(np.float16)  # [E, H, OC]
    wpv_t = np.ascontiguousarray(wpv.reshape(E, MT_H, 128, OC))
    b1_t = np.ascontiguousarray(b1.reshape(E, MT_H, 128).transpose(2, 0, 1))  # [128, E, MT_H]
    b2_t = np.ascontiguousarray(b2.reshape(E, MT_H, 128).transpose(2, 0, 1))
    bpv_t = np.ascontiguousarray(np.concatenate([bpi, bv[:, None]], axis=1))  # [E, OC]

    in_maps = []
    for r in range(NC):
        xs = xt16[:, r * ROWS : (r + 1) * ROWS]  # [D, ROWS]
        xh = np.ascontiguousarray(xs.reshape(KT_D, 128, ROWS).transpose(1, 0, 2))
        in_maps.append(
            {
                "xh": xh,
                "wr": wr_t,
                "w1": w1_t,
                "w2": w2_t,
                "wpv": wpv_t,
                "b1": b1_t,
                "b2": b2_t,
                "bpv": bpv_t,
            }
        )

    global _last_in_maps
    _last_in_maps = in_maps
    try:
        res = run_bass_kernel_spmd(nc, in_maps, core_ids=list(range(NC)))
    except Exception:
        # the axon/PJRT terminal occasionally reports a transient
        # NRT_EXEC_UNIT_UNRECOVERABLE on a cold session; one retry clears it
        res = run_bass_kernel_spmd(nc, in_maps, core_ids=list(range(NC)))
    pv = np.concatenate([res.results[r]["out"] for r in range(NC)], axis=0)  # [B, OC]
    w = np.concatenate([res.results[r]["w"] for r in range(NC)], axis=0)     # [B, E]
    pi = np.ascontiguousarray(pv[:, :A])
    value = np.ascontiguousarray(pv[:, A])
    return pi, value, w


# revision 28
# speedup vs baseline: 1.4785x; 1.4764x over previous
"""ActorCritic MoE (B=4096, D=512, H=1024, A=128, E=8, K=2) on 8 TRN2 NeuronCores.

Data-parallel: core r owns batch rows [r*512, (r+1)*512) end-to-end —
router + all 8 experts + the weighted combine — so there is no
collective at all; expert weights (3.3MB/expert fp16) stream through
SBUF double-buffered across the expert loop, spread over the 3
DMA-capable queues (sync/scalar/gpsimd).

Per-core dataflow (activations feature-major = "transposed"), all
matmuls fp16 in / fp32 accumulate (fp16's 11-bit mantissa keeps the
top-2 selection identical to fp32 on these inputs — verified: min
top2/top3 logit gap 3.5e-4 vs ~1e-4 logit error):
  router: logits[b,e] = xT_tile.T @ Wr, softmax/top-2 in exp-space
          (the softmax denominator cancels after renorm)
  per expert e:
    L1: h1T[h, b] = relu(W1e.T @ xT + b1e)
    L2: h2T[g, b] = relu(W2e.T @ h1T + b2e)
    L3: acc[b, 0:129] += (h2T_tile.T @ [Wpi|Wv]e + bpve) * w[:, e]
Outputs: pi [B,A], value [B], w [B,E].
"""

import numpy as np

import concourse.bass as bass
import concourse.mybir as mybir
import concourse.tile as tile
from concourse import bacc
from concourse.bass_utils import run_bass_kernel_spmd

B, D, H, A, E = 4096, 512, 1024, 128, 8
NC = 8
ROWS = B // NC              # batch rows per core
BT = ROWS // 128            # 4 b-tiles
KT_D = D // 128             # 4 k-tiles over D
MT_H = H // 128             # 8 tiles over H
OC = A + 1                  # 129 output columns (pi | value)

F32 = mybir.dt.float32
F16 = mybir.dt.float16
AF = mybir.ActivationFunctionType
ALU = mybir.AluOpType
AXL = mybir.AxisListType


def build():
    nc = bacc.Bacc("TRN2", target_bir_lowering=False, debug=False, num_devices=NC)

    # all inputs host-pretiled so every DMA is a contiguous block
    xh_e = nc.dram_tensor("xh", [128, KT_D, ROWS], F16, kind="ExternalInput")
    wr_e = nc.dram_tensor("wr", [128, KT_D, E], F16, kind="ExternalInput")
    w1_e = nc.dram_tensor("w1", [E, KT_D, 128, H], F16, kind="ExternalInput")
    w2_e = nc.dram_tensor("w2", [E, MT_H, 128, H], F16, kind="ExternalInput")
    wpv_e = nc.dram_tensor("wpv", [E, MT_H, 128, OC], F16, kind="ExternalInput")
    b1_e = nc.dram_tensor("b1", [128, E, MT_H], F32, kind="ExternalInput")
    b2_e = nc.dram_tensor("b2", [128, E, MT_H], F32, kind="ExternalInput")
    bpv_e = nc.dram_tensor("bpv", [E, OC], F32, kind="ExternalInput")

    out_e = nc.dram_tensor("out", [ROWS, OC], F32, kind="ExternalOutput")
    wout_e = nc.dram_tensor("w", [ROWS, E], F32, kind="ExternalOutput")

    with tile.TileContext(nc, pool_alloc_mode="queue") as tc:
        with (
            tc.tile_pool(name="wres", bufs=1) as wres,
            tc.tile_pool(name="wstream", bufs=2) as wstream,
            tc.tile_pool(name="acts", bufs=2) as acts,
            tc.tile_pool(name="rout", bufs=1) as rout,
            tc.tile_pool(name="outp", bufs=1) as outp,
            tc.tile_pool(name="ps", bufs=8, space="PSUM") as ps,
        ):
            # ---- one-time loads (fixed queue plan: what expert 0 needs first) ----
            wr_sb = wres.tile([128, KT_D, E], F16, tag="wr")
            nc.sync.dma_start(out=wr_sb[:], in_=wr_e.ap())
            xh = wres.tile([128, KT_D, ROWS], F16, tag="xh")
            xq = [nc.sync, nc.sync, nc.scalar, nc.gpsimd]
            for k in range(KT_D):
                xq[k].dma_start(out=xh[:, k, :], in_=xh_e.ap()[:, k, :])

            # expert-weight stream: double-buffered tiles, DMAs spread over
            # the three queues (per expert: sync 1.26MB, scalar 1MB, gpsimd 1MB)
            def load_expert(e):
                w1k = [
                    wstream.tile([128, H], F16, tag=f"w1k{k}", name=f"w1k{k}_{e}")
                    for k in range(KT_D)
                ]
                for k in range(KT_D):
                    eng = [nc.sync, nc.scalar, nc.gpsimd, nc.scalar][k]
                    eng.dma_start(out=w1k[k][:], in_=w1_e.ap()[e, k])
                w2k = [
                    wstream.tile([128, H], F16, tag=f"w2k{k}", name=f"w2k{k}_{e}")
                    for k in range(MT_H)
                ]
                for k in range(MT_H):
                    eng = [nc.scalar, nc.gpsimd, nc.sync][k % 3]
                    eng.dma_start(out=w2k[k][:], in_=w2_e.ap()[e, k])
                wpvk = [
                    wstream.tile([128, OC], F16, tag=f"wpvk{k}", name=f"wpvk{k}_{e}")
                    for k in range(MT_H)
                ]
                for k in range(MT_H):
                    eng = [nc.gpsimd, nc.sync][k % 2]
                    eng.dma_start(out=wpvk[k][:], in_=wpv_e.ap()[e, k])
                return w1k, w2k, wpvk

            wts = [load_expert(0)]
            b1_sb = wres.tile([128, E, MT_H], F32, tag="b1")
            nc.scalar.dma_start(out=b1_sb[:], in_=b1_e.ap())
            b2_sb = wres.tile([128, E, MT_H], F32, tag="b2")
            nc.scalar.dma_start(out=b2_sb[:], in_=b2_e.ap())
            bpv_ap = bpv_e.ap()
            bpv_sb = wres.tile([128, E, OC], F32, tag="bpv")
            nc.gpsimd.dma_start(
                out=bpv_sb[:],
                in_=bass.AP(
                    tensor=bpv_ap.tensor,
                    offset=bpv_ap.offset,
                    ap=[[0, 128], bpv_ap.ap[0], bpv_ap.ap[1]],
                ),
            )

            # ---- router: logits [128, 4, 8] ----
            lg = rout.tile([128, BT, E], F32, tag="lg")
            for t in range(BT):
                pr = ps.tile([128, E], F32, tag="ps")
                for k in range(KT_D):
                    nc.tensor.matmul(
                        pr[:],
                        lhsT=xh[:, k, t * 128 : (t + 1) * 128],
                        rhs=wr_sb[:, k, :],
                        start=(k == 0),
                        stop=(k == KT_D - 1),
                    )
                nc.scalar.activation(lg[:, t, :], pr[:], AF.Copy)

            # ---- softmax + top-2 + renormalize (exp-space) ----
            shp = [128, BT, E]
            m1 = rout.tile([128, BT], F32, tag="m1")
            nc.vector.tensor_reduce(m1[:], lg[:], axis=AXL.X, op=ALU.max)
            ex = rout.tile(shp, F32, tag="ex")
            nc.vector.tensor_tensor(ex[:], lg[:], m1[:].unsqueeze(-1).broadcast_to(shp), op=ALU.subtract)
            nc.scalar.activation(ex[:], ex[:], AF.Exp)
            t1 = rout.tile([128, BT], F32, tag="t1")
            nc.vector.tensor_reduce(t1[:], ex[:], axis=AXL.X, op=ALU.max)
            mask = rout.tile(shp, F32, tag="mask")
            nc.vector.tensor_tensor(mask[:], ex[:], t1[:].unsqueeze(-1).broadcast_to(shp), op=ALU.is_equal)
            e2 = rout.tile(shp, F32, tag="e2")
            nc.vector.scalar_tensor_tensor(e2[:], mask[:], -1.0, ex[:], op0=ALU.mult, op1=ALU.mult)
            nc.vector.tensor_tensor(e2[:], e2[:], ex[:], op=ALU.add)
            t2 = rout.tile([128, BT], F32, tag="t2")
            nc.vector.tensor_reduce(t2[:], e2[:], axis=AXL.X, op=ALU.max)
            mask2 = rout.tile(shp, F32, tag="mask2")
            nc.vector.tensor_tensor(mask2[:], e2[:], t2[:].unsqueeze(-1).broadcast_to(shp), op=ALU.is_equal)
            nc.vector.tensor_tensor(mask[:], mask[:], mask2[:], op=ALU.add)
            den = rout.tile([128, BT], F32, tag="den")
            nc.vector.tensor_tensor(den[:], t1[:], t2[:], op=ALU.add)
            rden = rout.tile([128, BT], F32, tag="rden")
            nc.vector.reciprocal(rden[:], den[:])
            wde = rout.tile(shp, F32, tag="wde")
            nc.vector.tensor_tensor(wde[:], ex[:], mask[:], op=ALU.mult)
            nc.vector.tensor_tensor(wde[:], wde[:], rden[:].unsqueeze(-1).broadcast_to(shp), op=ALU.mult)
            nc.scalar.dma_start(out=wout_e.ap().rearrange("(t p) e -> p t e", p=128), in_=wde[:])

            acc = [outp.tile([128, OC], F32, tag=f"acc{t}", name=f"acc{t}") for t in range(BT)]

            # ---- expert loop ----
            for e in range(E):
                if e + 1 < E:
                    wts.append(load_expert(e + 1))
                w1k, w2k, wpvk = wts[e]

                # L1
                h1t = acts.tile([128, MT_H, ROWS], F16, tag="h1t")
                for m in range(MT_H):
                    p1 = ps.tile([128, ROWS], F32, tag="ps")
                    for k in range(KT_D):
                        nc.tensor.matmul(
                            p1[:],
                            lhsT=w1k[k][:, m * 128 : (m + 1) * 128],
                            rhs=xh[:, k, :],
                            start=(k == 0),
                            stop=(k == KT_D - 1),
                        )
                    nc.scalar.activation(h1t[:, m, :], p1[:], AF.Relu, bias=b1_sb[:, e, m : m + 1])

                # L2
                h2t = acts.tile([128, MT_H, ROWS], F16, tag="h2t")
                for m in range(MT_H):
                    p2 = ps.tile([128, ROWS], F32, tag="ps")
                    for k in range(MT_H):
                        nc.tensor.matmul(
                            p2[:],
                            lhsT=w2k[k][:, m * 128 : (m + 1) * 128],
                            rhs=h1t[:, k, :],
                            start=(k == 0),
                            stop=(k == MT_H - 1),
                        )
                    nc.scalar.activation(h2t[:, m, :], p2[:], AF.Relu, bias=b2_sb[:, e, m : m + 1])

                # L3 + weighted accumulate into acc[t]
                for t in range(BT):
                    p3 = ps.tile([128, OC], F32, tag="ps")
                    for k in range(MT_H):
                        nc.tensor.matmul(
                            p3[:],
                            lhsT=h2t[:, k, t * 128 : (t + 1) * 128],
                            rhs=wpvk[k][:],
                            start=(k == 0),
                            stop=(k == MT_H - 1),
                        )
                    tmp = outp.tile([128, OC], F32, tag="tmp")
                    nc.vector.tensor_tensor(tmp[:], p3[:], bpv_sb[:, e, :], op=ALU.add)
                    if e == 0:
                        nc.vector.tensor_scalar_mul(acc[t][:], tmp[:], wde[:, t, e : e + 1])
                    else:
                        nc.vector.scalar_tensor_tensor(
                            acc[t][:], tmp[:], wde[:, t, e : e + 1], acc[t][:],
                            op0=ALU.mult, op1=ALU.add,
                        )
                    if e == E - 1:
                        nc.sync.dma_start(out=out_e.ap()[t * 128 : (t + 1) * 128, :], in_=acc[t][:])

    nc.compile()
    return nc


_NC_CACHE = None
_last_in_maps = None


def _get_nc():
    global _NC_CACHE
    if _NC_CACHE is None:
        _NC_CACHE = build()
    return _NC_CACHE


def kernel(x, Wr, W1, b1, W2, b2, Wpi, bpi, Wv, bv):
    x = np.ascontiguousarray(np.asarray(x, dtype=np.float32))
    Wr = np.ascontiguousarray(np.asarray(Wr, dtype=np.float32))
    W1 = np.asarray(W1, dtype=np.float32)
    b1 = np.asarray(b1, dtype=np.float32)
    W2 = np.asarray(W2, dtype=np.float32)
    b2 = np.asarray(b2, dtype=np.float32)
    Wpi = np.asarray(Wpi, dtype=np.float32)
    bpi = np.asarray(bpi, dtype=np.float32)
    Wv = np.asarray(Wv, dtype=np.float32)
    bv = np.asarray(bv, dtype=np.float32)

    nc = _get_nc()

    # shared (replicated) tensors, host-pretiled
    xt16 = x.T.astype(np.float16)  # [D, B]
    wr_t = np.ascontiguousarray(Wr.astype(np.float16).reshape(KT_D, 128, E).transpose(1, 0, 2))
    w1_t = np.ascontiguousarray(W1.astype(np.float16).reshape(E, KT_D, 128, H))
    w2_t = np.ascontiguousarray(W2.astype(np.float16).reshape(E, MT_H, 128, H))
    wpv = np.concatenate([Wpi, Wv[:, :, None]], axis=2).astype(np.float16)  # [E, H, OC]
    wpv_t = np.ascontiguousarray(wpv.reshape(E, MT_H, 128, OC))
    b1_t = np.ascontiguousarray(b1.reshape(E, MT_H, 128).transpose(2, 0, 1))  # [128, E, MT_H]
    b2_t = np.ascontiguousarray(b2.reshape(E, MT_H, 128).transpose(2, 0, 1))
    bpv_t = np.ascontiguousarray(np.concatenate([bpi, bv[:, None]], axis=1))  # [E, OC]

    in_maps = []
    for r in range(NC):
        xs = xt16[:, r * ROWS : (r + 1) * ROWS]  # [D, ROWS]
        xh = np.ascontiguousarray(xs.reshape(KT_D, 128, ROWS).transpose(1, 0, 2))
        in_maps.append(
            {
                "xh": xh,
                "wr": wr_t,
                "w1": w1_t,
                "w2": w2_t,
                "wpv": wpv_t,
                "b1": b1_t,
                "b2": b2_t,
                "bpv": bpv_t,
            }
        )

    global _last_in_maps
    _last_in_maps = in_maps
    try:
        res = run_bass_kernel_spmd(nc, in_maps, core_ids=list(range(NC)))
    except Exception:
        # the axon/PJRT terminal occasionally reports a transient
        # NRT_EXEC_UNIT_UNRECOVERABLE on a cold session; one retry clears it
        res = run_bass_kernel_spmd(nc, in_maps, core_ids=list(range(NC)))
    pv = np.concatenate([res.results[r]["out"] for r in range(NC)], axis=0)  # [B, OC]
    w = np.concatenate([res.results[r]["w"] for r in range(NC)], axis=0)     # [B, E]
    pi = np.ascontiguousarray(pv[:, :A])
    value = np.ascontiguousarray(pv[:, A])
    return pi, value, w
